# revision 11
# baseline (speedup 1.0000x reference)
"""Gated cosine-affinity kernel for Trainium2 (Bass/Tile), 8-core SPMD.

Problem: for each batch b (B=8):
    Xg = A_1 * X;  Yg = A_2 * Y            (elementwise gates)
    out[b] = normalize_rows(Xg) @ normalize_rows(Yg).T      (2048 x 2048)
with row norm = sqrt(max(|row|^2, 1e-6)).

Sharding: data-parallel over batch - one batch element per NeuronCore.

Perf design (vs ~59us fp32 DMA roofline of the naive layout):
  * inputs cast to fp16 on host (2MB/core), output quantized to uint8
    (cos in [-1,1]; u8 = 126*cos + 127.5, round-to-nearest on HW,
    dequant on host).  Per-core HBM traffic drops 21MB -> 6.3MB.
  * identity for PE transposes is host-uploaded (a gpsimd affine_select
    triggers an ~8us ucode LIBRARY_RELOAD stall on the engine that also
    does the gating).
  * matmul operands fp16 (PE streams 1 col/cycle @2.4GHz warm).  PE is
    warmed with dummy matmuls during the DMA head to beat the HAM clock
    gate (cold PE = 1.2GHz).
  * Y/A_2 host-block-permuted so the contiguous permuted load yields
    natural column order after the PE transpose; X's permutation is
    undone by the output store pattern instead.
  * stage 2 row-chunk-major: full 2048-wide uint8 rows per store
    (2KB/partition contiguous).  PSUM evac = fused scale+bias->uint8
    split ACT[0:1536] / DVE[1536:2048] per chunk - sized so ACT_total
    ~= DVE_total once DVE's head work (sum-squares etc) is counted.
  * op-count diet everywhere (engines have ~250ns/op floors): per-half
    gating, fused square+row-accum (scalar_tensor_tensor), one Newton
    1/sqrt chain per tensor with the 0.5 folded into the quant scale,
    yscale on gpsimd, 8 transposes batched per PSUM tile -> one evac.
  * DMA descriptor-gen spread over all three DGE paths: Y/A2 loads +
    even stores on Sync, X/A1/ident loads on ScalarE HWDGE, odd stores
    on GpSimd SWDGE.
"""

import numpy as np
from contextlib import ExitStack

import concourse.tile as tile
from concourse import bacc, mybir
from concourse.bass_utils import run_bass_kernel_spmd

B = 8
N = 2048          # rows of X (output rows)
M = 2048          # rows of Y (output cols)
D = 128           # feature dim == partition count == contraction dim
P = 128
EPS = 1e-6
# yn = yg * yinv2 * YSCALE with yinv2 = 1/(2*sqrt(vy)); evac scales by
# xinv2 = 1/(2*sqrt(vx)).  YSCALE = 4*126 makes the evac output 126*cos.
QSCALE = 126.0
YSCALE = 4.0 * QSCALE
QBIAS = 127.5
DEQ_OFF = 127.5   # HW float->uint8 convert rounds to nearest (measured)
NCH = N // P      # 16 row-chunks
H = NCH // 2

FP16 = mybir.dt.float16
FP32 = mybir.dt.float32
U8 = mybir.dt.uint8
AF = mybir.ActivationFunctionType
OP = mybir.AluOpType

ACT_COLS = 1024   # stage-2 evac split (psum-bank aligned)

_CACHED_NC = None


def _build_program():
    nc = bacc.Bacc("TRN2", target_bir_lowering=False, debug=False, num_devices=B)

    Xd = nc.dram_tensor("X", [N, D], FP16, kind="ExternalInput")
    Yd = nc.dram_tensor("Y", [M, D], FP16, kind="ExternalInput")
    A1d = nc.dram_tensor("A_1", [N, D], FP16, kind="ExternalInput")
    A2d = nc.dram_tensor("A_2", [M, D], FP16, kind="ExternalInput")
    IDd = nc.dram_tensor("I0", [P, P], FP16, kind="ExternalInput")
    OUT = nc.dram_tensor("out", [N, M], U8, kind="ExternalOutput")

    with tile.TileContext(nc) as tc, ExitStack() as ctx:
        consts = ctx.enter_context(tc.tile_pool(name="consts", bufs=1))
        raw = ctx.enter_context(tc.tile_pool(name="raw", bufs=1))
        gated = ctx.enter_context(tc.tile_pool(name="gated", bufs=1))
        small = ctx.enter_context(tc.tile_pool(name="small", bufs=1))
        sqscr = ctx.enter_context(tc.tile_pool(name="sqscr", bufs=2))
        tmat = ctx.enter_context(tc.tile_pool(name="tmat", bufs=1))
        ob_pool = ctx.enter_context(tc.tile_pool(name="ob", bufs=3))
        psum_t = ctx.enter_context(tc.tile_pool(name="psum_t", bufs=2, space="PSUM"))
        psum_a = ctx.enter_context(tc.tile_pool(name="psum_a", bufs=2, space="PSUM"))
        psum_b = ctx.enter_context(tc.tile_pool(name="psum_b", bufs=1, space="PSUM"))

        ident = consts.tile([P, P], FP16)
        warm1 = consts.tile([P, 1], FP32)

        # ---- loads: ident + X/A_1 via ScalarE HWDGE, Y/A_2 via Sync ----
        Xv = Xd.rearrange("(p c) d -> p c d", p=P)
        A1v = A1d.rearrange("(p c) d -> p c d", p=P)
        Yv = Yd.rearrange("(p c) d -> p c d", p=P)
        A2v = A2d.rearrange("(p c) d -> p c d", p=P)
        xraw = raw.tile([P, NCH, D], FP16, tag="x_raw")
        a1raw = raw.tile([P, NCH, D], FP16, tag="x_araw")
        yraw = raw.tile([P, NCH, D], FP16, tag="y_raw")
        a2raw = raw.tile([P, NCH, D], FP16, tag="y_araw")

        # ident + ACT sqrt-table warm on ScalarE first (the table load
        # costs ~1.3us and must not sit in front of load descriptor-gen);
        # all the big loads go through Sync's HWDGE.
        nc.vector.memset(warm1, 1.0)
        nc.scalar.dma_start(out=ident, in_=IDd[:, :])
        nc.scalar.sqrt(warm1, warm1)
        nc.sync.dma_start(out=yraw[:, 0:H, :], in_=Yv[:, 0:H, :])
        nc.sync.dma_start(out=a2raw[:, 0:H, :], in_=A2v[:, 0:H, :])
        nc.sync.dma_start(out=xraw[:, 0:4, :], in_=Xv[:, 0:4, :])
        nc.sync.dma_start(out=a1raw[:, 0:4, :], in_=A1v[:, 0:4, :])
        nc.sync.dma_start(out=yraw[:, H:NCH, :], in_=Yv[:, H:NCH, :])
        nc.sync.dma_start(out=a2raw[:, H:NCH, :], in_=A2v[:, H:NCH, :])
        nc.sync.dma_start(out=xraw[:, 4:NCH, :], in_=Xv[:, 4:NCH, :])
        nc.sync.dma_start(out=a1raw[:, 4:NCH, :], in_=A1v[:, 4:NCH, :])

        # PE warmup: ~14 back-to-back matmuls give ~3us sustained PE busy
        # during the DMA head, flipping the HAM clock gate to 2.4GHz.
        pwarm = psum_a.tile([P, ACT_COLS], FP32, tag="pa")
        for _ in range(14):
            nc.tensor.matmul(
                pwarm[:, 0:P], lhsT=ident, rhs=ident, start=True, stop=True
            )

        yg = gated.tile([P, NCH, D], FP16, tag="y_g")
        yn = gated.tile([P, NCH, D], FP16, tag="y_n")
        xg = gated.tile([P, NCH, D], FP16, tag="x_g")
        YnT = tmat.tile([P, M], FP16, tag="YnT")
        XgT = tmat.tile([P, N], FP16, tag="XgT")
        ysums = small.tile([P, NCH], FP32, tag="y_sums")
        xsums = small.tile([P, NCH], FP32, tag="x_sums")

        # X gating on GpSimd (off the DVE critical path)
        for q in range(4):
            sl = slice(4 * q, 4 * q + 4)
            nc.gpsimd.tensor_mul(xg[:, sl, :], xraw[:, sl, :], a1raw[:, sl, :])

        def sumsq(g_ap, sums_col):
            """Row sum-of-squares of one [128,128] chunk in ONE DVE op."""
            sq = sqscr.tile([P, D], FP16, tag="sq")
            nc.vector.scalar_tensor_tensor(
                out=sq, in0=g_ap, scalar=1.0, in1=g_ap,
                op0=OP.bypass, op1=OP.mult, accum_out=sums_col,
            )

        def rownorm_inv2(sums_ap, name, width):
            """inv2 = 1/(2*sqrt(max(sums, EPS))): ACT sqrt + Newton step,
            with the usual 0.5 factor folded into the caller's scale."""
            v = small.tile([P, width], FP32, tag=f"{name}_v")
            s = small.tile([P, width], FP32, tag=f"{name}_s")
            r = small.tile([P, width], FP32, tag=f"{name}_r")
            t = small.tile([P, width], FP32, tag=f"{name}_t")
            inv2 = small.tile([P, width], FP32, tag=f"{name}_i")
            nc.vector.tensor_scalar_max(v, sums_ap, EPS)
            nc.scalar.sqrt(s, v)
            nc.vector.reciprocal(r, s)
            nc.vector.tensor_mul(t, v, r)           # t = v/s
            nc.vector.tensor_add(t, t, s)           # t = s + v/s = 2*sqrt(v)
            nc.vector.reciprocal(inv2, t)
            return inv2

        # ---- Y prep ----------------------------------------------------
        # per half: gate (1 DVE op), 8 fused sum-squares; after both
        # halves one Newton chain; yscale on gpsimd; transposes batched
        # 8 per PSUM tile with a single DVE evac each.
        for h in range(2):
            sl = slice(H * h, H * (h + 1))
            nc.vector.tensor_mul(yg[:, sl, :], yraw[:, sl, :], a2raw[:, sl, :])
            for c in range(H * h, H * (h + 1)):
                sumsq(yg[:, c, :], ysums[:, c : c + 1])
            yinv2_h = rownorm_inv2(ysums[:, sl], f"y{h}", H)
            for k in range(H):
                c = H * h + k
                nc.vector.tensor_scalar(
                    out=yn[:, c, :], in0=yg[:, c, :],
                    scalar1=yinv2_h[:, k : k + 1], scalar2=YSCALE,
                    op0=OP.mult, op1=OP.mult,
                )
            pt = psum_t.tile([P, H * P], FP16, tag="pt")
            for k in range(H):
                c = H * h + k
                nc.tensor.transpose(pt[:, k * P : (k + 1) * P], yn[:, c, :], ident)
            nc.vector.tensor_copy(YnT[:, h * H * P : (h + 1) * H * P], pt)

        # ---- X prep ----------------------------------------------------
        for c in range(NCH):
            sumsq(xg[:, c, :], xsums[:, c : c + 1])
        xinv2 = rownorm_inv2(xsums, "x", NCH)

        def x_transpose_half(h):
            pt = psum_t.tile([P, H * P], FP16, tag="pt")
            for k in range(H):
                c = H * h + k
                nc.tensor.transpose(pt[:, k * P : (k + 1) * P], xg[:, c, :], ident)
            nc.vector.tensor_copy(XgT[:, h * H * P : (h + 1) * H * P], pt)

        x_transpose_half(0)

        # ---- stage 2 ---------------------------------------------------
        # Per row-chunk c: 4 matmuls into pmA(3 banks)+pmB(1 bank), evac
        # uint8 = psum*xinv2 + 127.5 with ACT on pmA, DVE on pmB, store
        # the full 2048-wide row group.
        OUTv = OUT.rearrange("(p s) m -> p s m", s=NCH)

        def stage2(c):
            lhsT = XgT[:, c * P : (c + 1) * P]
            pmA = psum_a.tile([P, ACT_COLS], FP32, tag="pa")
            pmB = psum_b.tile([P, M - ACT_COLS], FP32, tag="pb")
            for j in range(ACT_COLS // 512):
                nc.tensor.matmul(
                    pmA[:, j * 512 : (j + 1) * 512],
                    lhsT=lhsT,
                    rhs=YnT[:, j * 512 : (j + 1) * 512],
                    start=True, stop=True,
                )
            for j in range((M - ACT_COLS) // 512):
                nc.tensor.matmul(
                    pmB[:, j * 512 : (j + 1) * 512],
                    lhsT=lhsT,
                    rhs=YnT[:, ACT_COLS + j * 512 : ACT_COLS + (j + 1) * 512],
                    start=True, stop=True,
                )
            ob = ob_pool.tile([P, M], U8, tag="ob")
            xiv = xinv2[:, c : c + 1]
            nc.scalar.activation(
                ob[:, 0:ACT_COLS], pmA, AF.Copy, bias=QBIAS, scale=xiv
            )
            nc.vector.tensor_scalar(
                out=ob[:, ACT_COLS:M], in0=pmB,
                scalar1=xiv, scalar2=QBIAS, op0=OP.mult, op1=OP.add,
            )
            if c % 2 == 0:
                nc.sync.dma_start(out=OUTv[:, c, :], in_=ob)
            else:
                nc.gpsimd.dma_start(out=OUTv[:, c, :], in_=ob)

        for c in range(0, 8):
            stage2(c)
        x_transpose_half(1)
        for c in range(8, 16):
            stage2(c)

    nc.compile()
    return nc


def _get_program():
    global _CACHED_NC
    if _CACHED_NC is None:
        _CACHED_NC = _build_program()
    return _CACHED_NC


def _prep_xlike(a):
    # fp16 cast; device loads rows in permuted order (16p+c) which the
    # output store pattern undoes.
    return np.ascontiguousarray(a.astype(np.float16))


def _prep_ylike(a):
    # Host block-permute: device DRAM row 16p+c must hold natural row
    # c*128+p so transposed chunks come out in natural column order.
    return np.ascontiguousarray(
        a.reshape(NCH, P, D).transpose(1, 0, 2).reshape(M, D).astype(np.float16)
    )


_IDENT = np.eye(P, dtype=np.float16)


def kernel(X, Y, A_1, A_2, _trace=False, _trace_kwargs=None):
    X = np.asarray(X, dtype=np.float32)
    Y = np.asarray(Y, dtype=np.float32)
    A_1 = np.asarray(A_1, dtype=np.float32)
    A_2 = np.asarray(A_2, dtype=np.float32)
    assert X.shape == (B, N, D), X.shape

    nc = _get_program()
    in_maps = [
        {
            "X": _prep_xlike(X[b]),
            "Y": _prep_ylike(Y[b]),
            "A_1": _prep_xlike(A_1[b]),
            "A_2": _prep_ylike(A_2[b]),
            "I0": _IDENT,
        }
        for b in range(B)
    ]
    res = run_bass_kernel_spmd(
        nc,
        in_maps,
        list(range(B)),
        trace=_trace,
        **(_trace_kwargs or {}),
    )
    out = np.stack(
        [
            (res.results[b]["out"].astype(np.float32) - DEQ_OFF) * (1.0 / QSCALE)
            for b in range(B)
        ],
        axis=0,
    )
    if _trace:
        return out, res
    return out


# revision 12
# speedup vs baseline: 1.0633x; 1.0633x over previous
"""Gated cosine-affinity kernel for Trainium2 (Bass/Tile), 8-core SPMD.

Problem: for each batch b (B=8):
    Xg = A_1 * X;  Yg = A_2 * Y            (elementwise gates)
    out[b] = normalize_rows(Xg) @ normalize_rows(Yg).T      (2048 x 2048)
with row norm = sqrt(max(|row|^2, 1e-6)).

Sharding: data-parallel over batch - one batch element per NeuronCore.

Perf design notes:
  * inputs fp16 (host cast), output uint8 (u8 = 126*cos + 127.5, HW
    rounds to nearest; host dequant).  21MB -> 6.3MB HBM per core.
  * ONE psum ring: 2 x [128,2048] fp32 tiles (8 banks).  Used in turn by
    the PE warmup, the transpose groups (tiles bitcast to fp16), and
    stage 2 (one tile per row-chunk) - so stage 2 gets full double
    buffering AND an arbitrary ACT/DVE evac column split.
  * emission order keeps each engine FIFO stall-free: both Y halves'
    gates (DVE) + sum-squares (ACT) precede the norm chains; transpose
    evacs Y->ACT, X->DVE; X sum-squares (fused scalar_tensor_tensor)
    on DVE after the Y chain ops.
  * PE HAM clock gate: ~36 dummy matmuls in the DMA head + a re-warm
    burst right before stage 2 (an idle PE drops to 1.2GHz; stage-2
    matmuls measured ~390ns cold vs ~215ns warm).
  * stores alternate Sync/GpSimd DGE so descriptor-gen never queues
    behind one engine.
"""

import numpy as np
from contextlib import ExitStack

import concourse.tile as tile
from concourse import bacc, mybir
from concourse.bass_utils import run_bass_kernel_spmd

B = 8
N = 2048
M = 2048
D = 128
P = 128
EPS = 1e-6
QSCALE = 126.0
YSCALE = 4.0 * QSCALE   # yn = yg * [1/(2 sqrt(vy))] * YSCALE; evac * 1/(2 sqrt(vx))
QBIAS = 127.5
DEQ_OFF = 127.5         # HW float->uint8 rounds to nearest (measured)
NCH = N // P
H = NCH // 2

FP16 = mybir.dt.float16
FP32 = mybir.dt.float32
U8 = mybir.dt.uint8
AF = mybir.ActivationFunctionType
OP = mybir.AluOpType

ACT_COLS = 1152   # stage-2 evac split (free: reads may cross banks in-tile)

_CACHED_NC = None


def _build_program():
    nc = bacc.Bacc("TRN2", target_bir_lowering=False, debug=False, num_devices=B)

    Xd = nc.dram_tensor("X", [N, D], FP16, kind="ExternalInput")
    Yd = nc.dram_tensor("Y", [M, D], FP16, kind="ExternalInput")
    A1d = nc.dram_tensor("A_1", [N, D], FP16, kind="ExternalInput")
    A2d = nc.dram_tensor("A_2", [M, D], FP16, kind="ExternalInput")
    IDd = nc.dram_tensor("I0", [P, P], FP16, kind="ExternalInput")
    OUT = nc.dram_tensor("out", [N, M], U8, kind="ExternalOutput")

    with tile.TileContext(nc) as tc, ExitStack() as ctx:
        consts = ctx.enter_context(tc.tile_pool(name="consts", bufs=1))
        raw = ctx.enter_context(tc.tile_pool(name="raw", bufs=1))
        gated = ctx.enter_context(tc.tile_pool(name="gated", bufs=1))
        small = ctx.enter_context(tc.tile_pool(name="small", bufs=1))
        sqscr = ctx.enter_context(tc.tile_pool(name="sqscr", bufs=2))
        tmat = ctx.enter_context(tc.tile_pool(name="tmat", bufs=1))
        ob_pool = ctx.enter_context(tc.tile_pool(name="ob", bufs=3))
        psum = ctx.enter_context(tc.tile_pool(name="psum", bufs=2, space="PSUM"))

        ident = consts.tile([P, P], FP16)
        warm1 = consts.tile([P, 1], FP32)

        # ident + ACT sqrt-table warm first on ScalarE; big loads on Sync.
        nc.vector.memset(warm1, 1.0)
        nc.scalar.dma_start(out=ident, in_=IDd[:, :])
        nc.scalar.sqrt(warm1, warm1)

        Xv = Xd.rearrange("(p c) d -> p c d", p=P)
        A1v = A1d.rearrange("(p c) d -> p c d", p=P)
        Yv = Yd.rearrange("(p c) d -> p c d", p=P)
        A2v = A2d.rearrange("(p c) d -> p c d", p=P)
        xraw = raw.tile([P, NCH, D], FP16, tag="x_raw")
        a1raw = raw.tile([P, NCH, D], FP16, tag="x_araw")
        yraw = raw.tile([P, NCH, D], FP16, tag="y_raw")
        a2raw = raw.tile([P, NCH, D], FP16, tag="y_araw")

        nc.sync.dma_start(out=yraw[:, 0:H, :], in_=Yv[:, 0:H, :])
        nc.sync.dma_start(out=a2raw[:, 0:H, :], in_=A2v[:, 0:H, :])
        nc.sync.dma_start(out=xraw[:, 0:4, :], in_=Xv[:, 0:4, :])
        nc.sync.dma_start(out=a1raw[:, 0:4, :], in_=A1v[:, 0:4, :])
        nc.sync.dma_start(out=yraw[:, H:NCH, :], in_=Yv[:, H:NCH, :])
        nc.sync.dma_start(out=a2raw[:, H:NCH, :], in_=A2v[:, H:NCH, :])
        nc.sync.dma_start(out=xraw[:, 4:NCH, :], in_=Xv[:, 4:NCH, :])
        nc.sync.dma_start(out=a1raw[:, 4:NCH, :], in_=A1v[:, 4:NCH, :])

        yg = gated.tile([P, NCH, D], FP16, tag="y_g")
        yn = gated.tile([P, NCH, D], FP16, tag="y_n")
        xg = gated.tile([P, NCH, D], FP16, tag="x_g")
        YnT = tmat.tile([P, M], FP16, tag="YnT")
        XgT = tmat.tile([P, N], FP16, tag="XgT")
        ysums = small.tile([P, NCH], FP32, tag="y_sums")
        xsums = small.tile([P, NCH], FP32, tag="x_sums")

        # X gating on GpSimd (off the DVE critical path)
        for q in range(4):
            sl = slice(4 * q, 4 * q + 4)
            nc.gpsimd.tensor_mul(xg[:, sl, :], xraw[:, sl, :], a1raw[:, sl, :])

        # PE warmup vs the HAM clock gate: ~3.9us of sustained matmuls.
        pwarm = psum.tile([P, M], FP32, tag="pm")
        for _ in range(36):
            nc.tensor.matmul(
                pwarm[:, 0:P], lhsT=ident, rhs=ident, start=True, stop=True
            )

        # ---- Y gates (DVE) + sum-squares (ACT Square w/ row-accum) -----
        for h in range(2):
            sl = slice(H * h, H * (h + 1))
            nc.vector.tensor_mul(yg[:, sl, :], yraw[:, sl, :], a2raw[:, sl, :])
            for c in range(H * h, H * (h + 1)):
                sq = sqscr.tile([P, D], FP16, tag="sqa")
                nc.scalar.activation(
                    sq, yg[:, c, :], AF.Square, accum_out=ysums[:, c : c + 1]
                )

        def rownorm_inv2(sums_ap, name, width):
            """inv2 = 1/(2*sqrt(max(sums, EPS))): ACT sqrt + Newton step,
            the 0.5 folded into the caller's scale."""
            v = small.tile([P, width], FP32, tag=f"{name}_v")
            s = small.tile([P, width], FP32, tag=f"{name}_s")
            r = small.tile([P, width], FP32, tag=f"{name}_r")
            t = small.tile([P, width], FP32, tag=f"{name}_t")
            inv2 = small.tile([P, width], FP32, tag=f"{name}_i")
            nc.vector.tensor_scalar_max(v, sums_ap, EPS)
            nc.scalar.sqrt(s, v)
            nc.vector.reciprocal(r, s)
            nc.vector.tensor_mul(t, v, r)
            nc.vector.tensor_add(t, t, s)           # 2*sqrt(v)
            nc.vector.reciprocal(inv2, t)
            return inv2

        # ---- Y norm chains + scale (DVE) -------------------------------
        for h in range(2):
            sl = slice(H * h, H * (h + 1))
            yinv2_h = rownorm_inv2(ysums[:, sl], f"y{h}", H)
            for k in range(H):
                c = H * h + k
                nc.vector.tensor_scalar(
                    out=yn[:, c, :], in0=yg[:, c, :],
                    scalar1=yinv2_h[:, k : k + 1], scalar2=YSCALE,
                    op0=OP.mult, op1=OP.mult,
                )

        # ---- transposes: 8 per psum ring tile (bitcast fp16) -----------
        def transpose_half(src, h, dstT, evac_engine):
            ptile = psum.tile([P, M], FP32, tag="pm")
            pt16 = ptile.bitcast(FP16)
            for k in range(H):
                c = H * h + k
                nc.tensor.transpose(
                    pt16[:, k * P : (k + 1) * P], src[:, c, :], ident
                )
            if evac_engine == "act":
                nc.scalar.copy(dstT[:, h * H * P : (h + 1) * H * P], pt16[:, 0 : H * P])
            else:
                nc.vector.tensor_copy(
                    dstT[:, h * H * P : (h + 1) * H * P], pt16[:, 0 : H * P]
                )

        transpose_half(yn, 0, YnT, "act")
        transpose_half(yn, 1, YnT, "act")

        # ---- X sum-squares (fused DVE) + chain + transposes ------------
        for c in range(NCH):
            sq = sqscr.tile([P, D], FP16, tag="sqv")
            nc.vector.scalar_tensor_tensor(
                out=sq, in0=xg[:, c, :], scalar=1.0, in1=xg[:, c, :],
                op0=OP.bypass, op1=OP.mult, accum_out=xsums[:, c : c + 1],
            )
        xinv2 = rownorm_inv2(xsums, "x", NCH)
        transpose_half(xg, 0, XgT, "dve")
        transpose_half(xg, 1, XgT, "dve")

        # ---- stage 2 ---------------------------------------------------
        OUTv = OUT.rearrange("(p s) m -> p s m", s=NCH)

        def stage2(c, prewarm=0):
            lhsT = XgT[:, c * P : (c + 1) * P]
            pm = psum.tile([P, M], FP32, tag="pm")
            # re-warm the HAM right before stage 2: PE would otherwise
            # idle here waiting on the head and drop back to 1.2GHz
            for _ in range(prewarm):
                nc.tensor.matmul(
                    pm[:, 0:P], lhsT=ident, rhs=ident, start=True, stop=True
                )
            for j in range(4):
                nc.tensor.matmul(
                    pm[:, j * 512 : (j + 1) * 512],
                    lhsT=lhsT,
                    rhs=YnT[:, j * 512 : (j + 1) * 512],
                    start=True, stop=True,
                )
            ob = ob_pool.tile([P, M], U8, tag="ob")
            xiv = xinv2[:, c : c + 1]
            nc.scalar.activation(
                ob[:, 0:ACT_COLS], pm[:, 0:ACT_COLS], AF.Copy,
                bias=QBIAS, scale=xiv,
            )
            nc.vector.tensor_scalar(
                out=ob[:, ACT_COLS:M], in0=pm[:, ACT_COLS:M],
                scalar1=xiv, scalar2=QBIAS, op0=OP.mult, op1=OP.add,
            )
            if c % 2 == 0:
                nc.sync.dma_start(out=OUTv[:, c, :], in_=ob)
            else:
                nc.gpsimd.dma_start(out=OUTv[:, c, :], in_=ob)

        stage2(0, prewarm=20)
        for c in range(1, 16):
            stage2(c)

    nc.compile()
    return nc


def _get_program():
    global _CACHED_NC
    if _CACHED_NC is None:
        _CACHED_NC = _build_program()
    return _CACHED_NC


def _prep_xlike(a):
    return np.ascontiguousarray(a.astype(np.float16))


def _prep_ylike(a):
    # Host block-permute: device DRAM row 16p+c holds natural row c*128+p
    # so transposed chunks come out in natural column order.
    return np.ascontiguousarray(
        a.reshape(NCH, P, D).transpose(1, 0, 2).reshape(M, D).astype(np.float16)
    )


_IDENT = np.eye(P, dtype=np.float16)


def kernel(X, Y, A_1, A_2, _trace=False, _trace_kwargs=None):
    X = np.asarray(X, dtype=np.float32)
    Y = np.asarray(Y, dtype=np.float32)
    A_1 = np.asarray(A_1, dtype=np.float32)
    A_2 = np.asarray(A_2, dtype=np.float32)
    assert X.shape == (B, N, D), X.shape

    nc = _get_program()
    in_maps = [
        {
            "X": _prep_xlike(X[b]),
            "Y": _prep_ylike(Y[b]),
            "A_1": _prep_xlike(A_1[b]),
            "A_2": _prep_ylike(A_2[b]),
            "I0": _IDENT,
        }
        for b in range(B)
    ]
    res = run_bass_kernel_spmd(
        nc,
        in_maps,
        list(range(B)),
        trace=_trace,
        **(_trace_kwargs or {}),
    )
    out = np.stack(
        [
            (res.results[b]["out"].astype(np.float32) - DEQ_OFF) * (1.0 / QSCALE)
            for b in range(B)
        ],
        axis=0,
    )
    if _trace:
        return out, res
    return out


# revision 13
# speedup vs baseline: 1.1478x; 1.0795x over previous
"""Gated cosine-affinity kernel for Trainium2 (Bass/Tile), 8-core SPMD.

Problem: for each batch b (B=8):
    Xg = A_1 * X;  Yg = A_2 * Y            (elementwise gates)
    out[b] = normalize_rows(Xg) @ normalize_rows(Yg).T      (2048 x 2048)
with row norm = sqrt(max(|row|^2, 1e-6)).

Sharding: data-parallel over batch - one batch element per NeuronCore.

Perf design notes:
  * inputs fp16 (host cast), output uint8 (u8 = 126*cos + 127.5, HW
    rounds to nearest; host dequant).  21MB -> 6.3MB HBM per core.
  * ONE psum ring: 8 x [128,512] fp32 tiles (one bank each).  The PE
    warmup, the transpose groups (tiles bitcast to [128,1024] fp16, 8
    transposes per tile), and stage 2 all allocate from it.  Stage 2 is
    STRIP-granular: each 512-col matmul -> its own evac -> slot free,
    so ~8 strips are in flight and the ~0.7us cross-engine semaphore
    hops hide behind the pipeline instead of gating a 2-deep ring.
  * emission order avoids FIFO head-of-line blocking: gates + all
    sum-squares are queued before the norm chains on each engine, and
    each chain's ACT sqrt is emitted between the two halves' work.
  * PE HAM clock gate: warmup matmuls in the DMA head + a re-warm burst
    right before stage 2 (idle PE drops to 1.2GHz; stage-2 matmuls
    measured ~390ns cold vs ~215ns warm).
  * evac engine split by global strip index (36 ACT / 28 DVE) to
    balance ACT vs DVE total runtime including DVE's head work.
"""

import numpy as np
from contextlib import ExitStack

import concourse.tile as tile
from concourse import bacc, mybir
from concourse.bass_utils import run_bass_kernel_spmd

B = 8
N = 2048
M = 2048
D = 128
P = 128
EPS = 1e-6
QSCALE = 126.0
YSCALE = 4.0 * QSCALE   # yn = yg * [1/(2 sqrt(vy))] * YSCALE; evac * 1/(2 sqrt(vx))
QBIAS = 127.5
DEQ_OFF = 127.5         # HW float->uint8 rounds to nearest (measured)
NCH = N // P
H = NCH // 2

FP16 = mybir.dt.float16
FP32 = mybir.dt.float32
U8 = mybir.dt.uint8
AF = mybir.ActivationFunctionType
OP = mybir.AluOpType

_CACHED_NC = None


def _build_program():
    nc = bacc.Bacc("TRN2", target_bir_lowering=False, debug=False, num_devices=B)

    Xd = nc.dram_tensor("X", [N, D], FP16, kind="ExternalInput")
    Yd = nc.dram_tensor("Y", [M, D], FP16, kind="ExternalInput")
    A1d = nc.dram_tensor("A_1", [N, D], FP16, kind="ExternalInput")
    A2d = nc.dram_tensor("A_2", [M, D], FP16, kind="ExternalInput")
    IDd = nc.dram_tensor("I0", [P, P], FP16, kind="ExternalInput")
    OUT = nc.dram_tensor("out", [N, M], U8, kind="ExternalOutput")

    with tile.TileContext(nc) as tc, ExitStack() as ctx:
        consts = ctx.enter_context(tc.tile_pool(name="consts", bufs=1))
        raw = ctx.enter_context(tc.tile_pool(name="raw", bufs=1))
        gated = ctx.enter_context(tc.tile_pool(name="gated", bufs=1))
        small = ctx.enter_context(tc.tile_pool(name="small", bufs=1))
        sqscr = ctx.enter_context(tc.tile_pool(name="sqscr", bufs=2))
        tmat = ctx.enter_context(tc.tile_pool(name="tmat", bufs=1))
        ob_pool = ctx.enter_context(tc.tile_pool(name="ob", bufs=3))
        psum = ctx.enter_context(tc.tile_pool(name="psum", bufs=8, space="PSUM"))

        ident = consts.tile([P, P], FP16)
        warm1 = consts.tile([P, 1], FP32)

        # ident + ACT sqrt-table warm first on ScalarE; big loads on Sync.
        nc.vector.memset(warm1, 1.0)
        nc.scalar.dma_start(out=ident, in_=IDd[:, :])
        nc.scalar.sqrt(warm1, warm1)

        Xv = Xd.rearrange("(p c) d -> p c d", p=P)
        A1v = A1d.rearrange("(p c) d -> p c d", p=P)
        Yv = Yd.rearrange("(p c) d -> p c d", p=P)
        A2v = A2d.rearrange("(p c) d -> p c d", p=P)
        xraw = raw.tile([P, NCH, D], FP16, tag="x_raw")
        a1raw = raw.tile([P, NCH, D], FP16, tag="x_araw")
        yraw = raw.tile([P, NCH, D], FP16, tag="y_raw")
        a2raw = raw.tile([P, NCH, D], FP16, tag="y_araw")

        # X needs its data earliest: its head chain (gate->sumsq->chain->
        # transpose->evac) is longer than Y's.
        nc.sync.dma_start(out=yraw[:, 0:H, :], in_=Yv[:, 0:H, :])
        nc.sync.dma_start(out=a2raw[:, 0:H, :], in_=A2v[:, 0:H, :])
        nc.sync.dma_start(out=xraw[:, 0:H, :], in_=Xv[:, 0:H, :])
        nc.sync.dma_start(out=a1raw[:, 0:H, :], in_=A1v[:, 0:H, :])
        nc.sync.dma_start(out=xraw[:, H:NCH, :], in_=Xv[:, H:NCH, :])
        nc.sync.dma_start(out=a1raw[:, H:NCH, :], in_=A1v[:, H:NCH, :])
        nc.sync.dma_start(out=yraw[:, H:NCH, :], in_=Yv[:, H:NCH, :])
        nc.sync.dma_start(out=a2raw[:, H:NCH, :], in_=A2v[:, H:NCH, :])

        yg = gated.tile([P, NCH, D], FP16, tag="y_g")
        yn = gated.tile([P, NCH, D], FP16, tag="y_n")
        xg = gated.tile([P, NCH, D], FP16, tag="x_g")
        YnT = tmat.tile([P, M], FP16, tag="YnT")
        XgT = tmat.tile([P, N], FP16, tag="XgT")
        ysums = small.tile([P, NCH], FP32, tag="y_sums")
        xsums = small.tile([P, NCH], FP32, tag="x_sums")

        # X gating on GpSimd (off the DVE critical path)
        for q in range(4):
            sl = slice(4 * q, 4 * q + 4)
            nc.gpsimd.tensor_mul(xg[:, sl, :], xraw[:, sl, :], a1raw[:, sl, :])

        # PE warmup vs the HAM clock gate (~3.9us sustained)
        pwarm = psum.tile([P, 512], FP32, tag="ps")
        for _ in range(36):
            nc.tensor.matmul(
                pwarm[:, 0:P], lhsT=ident, rhs=ident, start=True, stop=True
            )

        # ---- gates + ALL sum-squares queued before the chains ----------
        # DVE: Ygate h0, Ygate h1, 16 X fused sumsq; ACT: 16 Y Square+acc.
        nc.vector.tensor_mul(yg[:, 0:H, :], yraw[:, 0:H, :], a2raw[:, 0:H, :])
        for c in range(0, H):
            sq = sqscr.tile([P, D], FP16, tag="sqa")
            nc.scalar.activation(
                sq, yg[:, c, :], AF.Square, accum_out=ysums[:, c : c + 1]
            )
        nc.vector.tensor_mul(yg[:, H:NCH, :], yraw[:, H:NCH, :], a2raw[:, H:NCH, :])
        for c in range(NCH):
            sq = sqscr.tile([P, D], FP16, tag="sqv")
            nc.vector.scalar_tensor_tensor(
                out=sq, in0=xg[:, c, :], scalar=1.0, in1=xg[:, c, :],
                op0=OP.bypass, op1=OP.mult, accum_out=xsums[:, c : c + 1],
            )
        for c in range(H, NCH):
            sq = sqscr.tile([P, D], FP16, tag="sqa")
            nc.scalar.activation(
                sq, yg[:, c, :], AF.Square, accum_out=ysums[:, c : c + 1]
            )

        def rownorm_inv2(sums_ap, name, width):
            """inv2 = 1/(2*sqrt(max(sums, EPS))): ACT sqrt + Newton step,
            the 0.5 folded into the caller's scale."""
            v = small.tile([P, width], FP32, tag=f"{name}_v")
            s = small.tile([P, width], FP32, tag=f"{name}_s")
            r = small.tile([P, width], FP32, tag=f"{name}_r")
            t = small.tile([P, width], FP32, tag=f"{name}_t")
            inv2 = small.tile([P, width], FP32, tag=f"{name}_i")
            nc.vector.tensor_scalar_max(v, sums_ap, EPS)
            nc.scalar.sqrt(s, v)
            nc.vector.reciprocal(r, s)
            nc.vector.tensor_mul(t, v, r)
            nc.vector.tensor_add(t, t, s)           # 2*sqrt(v)
            nc.vector.reciprocal(inv2, t)
            return inv2

        # ---- norm chains + y scaling -----------------------------------
        yinv2_0 = rownorm_inv2(ysums[:, 0:H], "y0", H)
        for k in range(H):
            nc.vector.tensor_scalar(
                out=yn[:, k, :], in0=yg[:, k, :],
                scalar1=yinv2_0[:, k : k + 1], scalar2=YSCALE,
                op0=OP.mult, op1=OP.mult,
            )
        yinv2_1 = rownorm_inv2(ysums[:, H:NCH], "y1", H)
        for k in range(H):
            c = H + k
            nc.vector.tensor_scalar(
                out=yn[:, c, :], in0=yg[:, c, :],
                scalar1=yinv2_1[:, k : k + 1], scalar2=YSCALE,
                op0=OP.mult, op1=OP.mult,
            )
        xinv2 = rownorm_inv2(xsums, "x", NCH)

        # ---- transposes: 8 per 1-bank psum tile (bitcast fp16) ---------
        def transpose_half(src, h, dstT, evac_engine):
            ptile = psum.tile([P, 512], FP32, tag="ps")
            pt16 = ptile.bitcast(FP16)
            for k in range(H):
                c = H * h + k
                nc.tensor.transpose(
                    pt16[:, k * P : (k + 1) * P], src[:, c, :], ident
                )
            if evac_engine == "act":
                nc.scalar.copy(dstT[:, h * H * P : (h + 1) * H * P], pt16)
            else:
                nc.vector.tensor_copy(dstT[:, h * H * P : (h + 1) * H * P], pt16)

        transpose_half(yn, 0, YnT, "act")
        transpose_half(yn, 1, YnT, "act")
        transpose_half(xg, 0, XgT, "dve")
        transpose_half(xg, 1, XgT, "dve")

        # ---- stage 2: strip-granular pipeline --------------------------
        OUTv = OUT.rearrange("(p s) m -> p s m", s=NCH)

        # 36 of 64 strips on ACT, 28 on DVE (balances ACT vs DVE totals
        # including DVE's head work); spread evenly by global strip id.
        def strip_engine(s):
            return "act" if (s * 36) % 64 < 36 else "dve"

        def stage2(c, prewarm=0):
            lhsT = XgT[:, c * P : (c + 1) * P]
            ob = ob_pool.tile([P, M], U8, tag="ob")
            xiv = xinv2[:, c : c + 1]
            for j in range(4):
                ps = psum.tile([P, 512], FP32, tag="ps")
                if prewarm and j == 0:
                    for _ in range(prewarm):
                        nc.tensor.matmul(
                            ps[:, 0:P], lhsT=ident, rhs=ident,
                            start=True, stop=True,
                        )
                nc.tensor.matmul(
                    ps, lhsT=lhsT, rhs=YnT[:, j * 512 : (j + 1) * 512],
                    start=True, stop=True,
                )
                dst = ob[:, j * 512 : (j + 1) * 512]
                if strip_engine(4 * c + j) == "act":
                    nc.scalar.activation(dst, ps, AF.Copy, bias=QBIAS, scale=xiv)
                else:
                    nc.vector.tensor_scalar(
                        out=dst, in0=ps, scalar1=xiv, scalar2=QBIAS,
                        op0=OP.mult, op1=OP.add,
                    )
            if c % 2 == 0:
                nc.sync.dma_start(out=OUTv[:, c, :], in_=ob)
            else:
                nc.gpsimd.dma_start(out=OUTv[:, c, :], in_=ob)

        stage2(0, prewarm=18)
        for c in range(1, 16):
            stage2(c)

    nc.compile()
    return nc


def _get_program():
    global _CACHED_NC
    if _CACHED_NC is None:
        _CACHED_NC = _build_program()
    return _CACHED_NC


def _prep_xlike(a):
    return np.ascontiguousarray(a.astype(np.float16))


def _prep_ylike(a):
    # Host block-permute: device DRAM row 16p+c holds natural row c*128+p
    # so transposed chunks come out in natural column order.
    return np.ascontiguousarray(
        a.reshape(NCH, P, D).transpose(1, 0, 2).reshape(M, D).astype(np.float16)
    )


_IDENT = np.eye(P, dtype=np.float16)


def kernel(X, Y, A_1, A_2, _trace=False, _trace_kwargs=None):
    X = np.asarray(X, dtype=np.float32)
    Y = np.asarray(Y, dtype=np.float32)
    A_1 = np.asarray(A_1, dtype=np.float32)
    A_2 = np.asarray(A_2, dtype=np.float32)
    assert X.shape == (B, N, D), X.shape

    nc = _get_program()
    in_maps = [
        {
            "X": _prep_xlike(X[b]),
            "Y": _prep_ylike(Y[b]),
            "A_1": _prep_xlike(A_1[b]),
            "A_2": _prep_ylike(A_2[b]),
            "I0": _IDENT,
        }
        for b in range(B)
    ]
    res = run_bass_kernel_spmd(
        nc,
        in_maps,
        list(range(B)),
        trace=_trace,
        **(_trace_kwargs or {}),
    )
    out = np.stack(
        [
            (res.results[b]["out"].astype(np.float32) - DEQ_OFF) * (1.0 / QSCALE)
            for b in range(B)
        ],
        axis=0,
    )
    if _trace:
        return out, res
    return out


# revision 14
# speedup vs baseline: 1.3183x; 1.1485x over previous
"""v2 structure (best measured: 60.8us) + two targeted fixes:
ident via host upload (kills the gpsimd LIBRARY_RELOAD stall) and a PE
re-warm burst before stage 2 (stage-2 matmuls measured cold otherwise).
"""

import numpy as np
from contextlib import ExitStack

import concourse.tile as tile
from concourse import bacc, mybir
from concourse.bass_utils import run_bass_kernel_spmd

B = 8
N = 2048
M = 2048
D = 128
P = 128
EPS = 1e-6
QSCALE = 126.0
QBIAS = 127.5
DEQ_OFF = 127.5
NCH = N // P
H = NCH // 2

FP16 = mybir.dt.float16
FP32 = mybir.dt.float32
U8 = mybir.dt.uint8
AF = mybir.ActivationFunctionType
OP = mybir.AluOpType

_CACHED_NC = None


def _build_program():
    nc = bacc.Bacc("TRN2", target_bir_lowering=False, debug=False, num_devices=B)

    Xd = nc.dram_tensor("X", [N, D], FP16, kind="ExternalInput")
    Yd = nc.dram_tensor("Y", [M, D], FP16, kind="ExternalInput")
    A1d = nc.dram_tensor("A_1", [N, D], FP16, kind="ExternalInput")
    A2d = nc.dram_tensor("A_2", [M, D], FP16, kind="ExternalInput")
    IDd = nc.dram_tensor("I0", [P, P], FP16, kind="ExternalInput")
    OUT = nc.dram_tensor("out", [N, M], U8, kind="ExternalOutput")

    with tile.TileContext(nc) as tc, ExitStack() as ctx:
        consts = ctx.enter_context(tc.tile_pool(name="consts", bufs=1))
        raw = ctx.enter_context(tc.tile_pool(name="raw", bufs=1))
        gated = ctx.enter_context(tc.tile_pool(name="gated", bufs=1))
        small = ctx.enter_context(tc.tile_pool(name="small", bufs=1))
        sqscr = ctx.enter_context(tc.tile_pool(name="sqscr", bufs=2))
        tmat = ctx.enter_context(tc.tile_pool(name="tmat", bufs=1))
        ob_pool = ctx.enter_context(tc.tile_pool(name="ob", bufs=3))
        psum_t = ctx.enter_context(tc.tile_pool(name="psum_t", bufs=2, space="PSUM"))
        psum_mm = ctx.enter_context(tc.tile_pool(name="psum_mm", bufs=3, space="PSUM"))

        ident = consts.tile([P, P], FP16)
        warm1 = consts.tile([P, 1], FP32)

        nc.vector.memset(warm1, 1.0)
        nc.scalar.dma_start(out=ident, in_=IDd[:, :])
        nc.scalar.sqrt(warm1, warm1)

        Xv = Xd.rearrange("(p c) d -> p c d", p=P)
        A1v = A1d.rearrange("(p c) d -> p c d", p=P)
        Yv = Yd.rearrange("(p c) d -> p c d", p=P)
        A2v = A2d.rearrange("(p c) d -> p c d", p=P)
        xraw = raw.tile([P, NCH, D], FP16, tag="x_raw")
        a1raw = raw.tile([P, NCH, D], FP16, tag="x_araw")
        yraw = raw.tile([P, NCH, D], FP16, tag="y_raw")
        a2raw = raw.tile([P, NCH, D], FP16, tag="y_araw")

        nc.sync.dma_start(out=yraw[:, 0:H, :], in_=Yv[:, 0:H, :])
        nc.sync.dma_start(out=a2raw[:, 0:H, :], in_=A2v[:, 0:H, :])
        nc.sync.dma_start(out=xraw[:, 0:4, :], in_=Xv[:, 0:4, :])
        nc.sync.dma_start(out=a1raw[:, 0:4, :], in_=A1v[:, 0:4, :])
        nc.sync.dma_start(out=yraw[:, H:NCH, :], in_=Yv[:, H:NCH, :])
        nc.sync.dma_start(out=a2raw[:, H:NCH, :], in_=A2v[:, H:NCH, :])
        for q in range(1, 4):
            sl = slice(4 * q, 4 * q + 4)
            nc.sync.dma_start(out=xraw[:, sl, :], in_=Xv[:, sl, :])
            nc.sync.dma_start(out=a1raw[:, sl, :], in_=A1v[:, sl, :])

        # PE warmup
        pwarm = psum_mm.tile([P, 2 * 512], FP32, tag="pm")
        for _ in range(18):
            nc.tensor.matmul(
                pwarm[:, 0:P], lhsT=ident, rhs=ident, start=True, stop=True
            )

        yg = gated.tile([P, NCH, D], FP16, tag="y_g")
        yn = gated.tile([P, NCH, D], FP16, tag="y_n")
        xg = gated.tile([P, NCH, D], FP16, tag="x_g")
        YnT = tmat.tile([P, M], FP16, tag="YnT")
        XgT = tmat.tile([P, N], FP16, tag="XgT")
        ysums = small.tile([P, NCH], FP32, tag="y_sums")
        xsums = small.tile([P, NCH], FP32, tag="x_sums")

        for q in range(4):
            sl = slice(4 * q, 4 * q + 4)
            nc.gpsimd.tensor_mul(xg[:, sl, :], xraw[:, sl, :], a1raw[:, sl, :])

        def sumsq(g_ap, sums_col):
            sq = sqscr.tile([P, D], FP16, tag="sq")
            nc.vector.scalar_tensor_tensor(
                out=sq, in0=g_ap, scalar=1.0, in1=g_ap,
                op0=OP.bypass, op1=OP.mult, accum_out=sums_col,
            )

        def rownorm_inv(sums_ap, name, width):
            v = small.tile([P, width], FP32, tag=f"{name}_v")
            s = small.tile([P, width], FP32, tag=f"{name}_s")
            r = small.tile([P, width], FP32, tag=f"{name}_r")
            t = small.tile([P, width], FP32, tag=f"{name}_t")
            inv = small.tile([P, width], FP32, tag=f"{name}_inv")
            nc.vector.tensor_scalar_max(v, sums_ap, EPS)
            nc.scalar.sqrt(s, v)
            nc.vector.reciprocal(r, s)
            nc.vector.tensor_mul(t, v, r)
            nc.vector.tensor_add(t, t, s)
            nc.vector.tensor_scalar_mul(t, t, 0.5)
            nc.vector.reciprocal(inv, t)
            return inv

        def y_half(h):
            base = H * h
            for q2 in range(2):
                sl = slice(base + 4 * q2, base + 4 * q2 + 4)
                nc.vector.tensor_mul(yg[:, sl, :], yraw[:, sl, :], a2raw[:, sl, :])
                for k in range(4):
                    c = base + 4 * q2 + k
                    sumsq(yg[:, c, :], ysums[:, c : c + 1])
            yinv_h = rownorm_inv(ysums[:, base : base + H], f"y{h}", H)
            for k in range(H):
                c = base + k
                nc.vector.tensor_scalar(
                    out=yn[:, c, :], in0=yg[:, c, :],
                    scalar1=yinv_h[:, k : k + 1], scalar2=QSCALE,
                    op0=OP.mult, op1=OP.mult,
                )
            for g4 in range(2):
                pt = psum_t.tile([P, 4 * P], FP16, tag="pt")
                for k in range(4):
                    c = base + 4 * g4 + k
                    nc.tensor.transpose(pt[:, k * P : (k + 1) * P], yn[:, c, :], ident)
                c0 = base + 4 * g4
                nc.scalar.copy(YnT[:, c0 * P : (c0 + 4) * P], pt)

        xinv_q = [None] * 4

        def x_quarter(q):
            for k in range(4):
                c = 4 * q + k
                sumsq(xg[:, c, :], xsums[:, c : c + 1])
            xinv_q[q] = rownorm_inv(xsums[:, 4 * q : 4 * q + 4], f"x{q}", 4)
            pt = psum_t.tile([P, 4 * P], FP16, tag="pt")
            for k in range(4):
                c = 4 * q + k
                nc.tensor.transpose(pt[:, k * P : (k + 1) * P], xg[:, c, :], ident)
            c0 = 4 * q
            nc.vector.tensor_copy(XgT[:, c0 * P : (c0 + 4) * P], pt)

        OUTv = OUT.rearrange("(p s) m -> p s m", s=NCH)

        def stage2(c, prewarm=0):
            q, k = divmod(c, 4)
            lhsT = XgT[:, c * P : (c + 1) * P]
            pmA = psum_mm.tile([P, 2 * 512], FP32, tag="pm")
            pmB = psum_mm.tile([P, 2 * 512], FP32, tag="pm")
            if prewarm:
                for _ in range(prewarm):
                    nc.tensor.matmul(
                        pmA[:, 0:P], lhsT=ident, rhs=ident, start=True, stop=True
                    )
            for j in range(2):
                nc.tensor.matmul(
                    pmA[:, j * 512 : (j + 1) * 512],
                    lhsT=lhsT, rhs=YnT[:, j * 512 : (j + 1) * 512],
                    start=True, stop=True,
                )
            for j in range(2):
                nc.tensor.matmul(
                    pmB[:, j * 512 : (j + 1) * 512],
                    lhsT=lhsT, rhs=YnT[:, (j + 2) * 512 : (j + 3) * 512],
                    start=True, stop=True,
                )
            ob = ob_pool.tile([P, M], U8, tag="ob")
            xiv = xinv_q[q][:, k : k + 1]
            nc.scalar.activation(
                ob[:, 0:1024], pmA, AF.Copy, bias=QBIAS, scale=xiv
            )
            nc.vector.tensor_scalar(
                out=ob[:, 1024:2048], in0=pmB,
                scalar1=xiv, scalar2=QBIAS, op0=OP.mult, op1=OP.add,
            )
            nc.sync.dma_start(out=OUTv[:, c, :], in_=ob)

        y_half(0)
        x_quarter(0)
        y_half(1)
        x_quarter(1)
        stage2(0, prewarm=16)
        for c in range(1, 4):
            stage2(c)
        x_quarter(2)
        for c in range(4, 8):
            stage2(c)
        x_quarter(3)
        for c in range(8, 16):
            stage2(c)

    nc.compile()
    return nc


def _get_program():
    global _CACHED_NC
    if _CACHED_NC is None:
        _CACHED_NC = _build_program()
    return _CACHED_NC


def _prep_xlike(a):
    return np.ascontiguousarray(a.astype(np.float16))


def _prep_ylike(a):
    return np.ascontiguousarray(
        a.reshape(NCH, P, D).transpose(1, 0, 2).reshape(M, D).astype(np.float16)
    )


_IDENT = np.eye(P, dtype=np.float16)


def kernel(X, Y, A_1, A_2, _trace=False, _trace_kwargs=None):
    X = np.asarray(X, dtype=np.float32)
    Y = np.asarray(Y, dtype=np.float32)
    A_1 = np.asarray(A_1, dtype=np.float32)
    A_2 = np.asarray(A_2, dtype=np.float32)
    assert X.shape == (B, N, D), X.shape

    nc = _get_program()
    in_maps = [
        {
            "X": _prep_xlike(X[b]),
            "Y": _prep_ylike(Y[b]),
            "A_1": _prep_xlike(A_1[b]),
            "A_2": _prep_ylike(A_2[b]),
            "I0": _IDENT,
        }
        for b in range(B)
    ]
    res = run_bass_kernel_spmd(
        nc, in_maps, list(range(B)), trace=_trace, **(_trace_kwargs or {})
    )
    out = np.stack(
        [
            (res.results[b]["out"].astype(np.float32) - DEQ_OFF) * (1.0 / QSCALE)
            for b in range(B)
        ],
        axis=0,
    )
    if _trace:
        return out, res
    return out
